# revision 21
# baseline (speedup 1.0000x reference)
"""GraphSAGE 2-layer kernel for 8 Trainium2 NeuronCores.

Strategy (dst-partitioned, batched ucode gather + DVE select/fold reduction):
  - Nodes dealt round-robin by global in-degree rank to the 8 cores, so every
    core's rank-r row has ~the same degree -> per-block run widths are
    near-uniform and padding is small (~5%).
  - Pre-projection before the gather: p = h @ W_neigh per-core, AllGather'd
    to a bf16 [NPAD, 64] table in DRAM. The gather reads it through a
    [NPAD/2, 128] pair view: one 256B gather row = nodes (2q, 2q+1), so the
    int16 gather index q = src>>1 covers the whole table in one window.
  - Aggregation per block-group: one dma_gather pulls all edge tokens
    (token i -> partition i%128, column i//128; layout [128, nblk, G, 128]),
    a 3-op DVE select picks the src&1 half per token (f32 out), and a
    log-depth in-place strided DVE fold sums each row's G-token run, with the
    last step written straight into the agg columns. Pad tokens point at an
    all-zero pair row.
  - h = relu(x @ W_self + inv_deg * agg + b) with full-width in-place DVE ops
    and one scalar-engine relu; layer 2 identical (same graph -> same index
    streams) with an on-chip PE transpose of h1.
"""

import numpy as np

N = 50000
E = 800000
IN_F, HID_F, OUT_F = 128, 64, 64
CORES = 8
P = 128
NPC = N // CORES          # 6250 real nodes per core
NB_C = 49                 # dst blocks per core
R = NB_C * P              # 6272 padded rows per core
NPAD = CORES * R          # 50176
W = NB_C * HID_F          # 3136 full-width columns
ZQ = (NPC) // 2           # 3125: pair (6250, 6251) = core 0 pad rows, all-zero
CH_COLS = 100             # max token-columns per gather (12800 tokens)

_cache = {}


def _prep(x, src, dst):
    """Host-side: degree-deal nodes to cores, build grouped token streams."""
    src = np.asarray(src).astype(np.int64)
    dst = np.asarray(dst).astype(np.int64)
    deg = np.bincount(dst, minlength=N)

    order = np.argsort(-deg, kind="stable")
    pos = np.empty(N, np.int64)
    i = np.arange(N)
    pos[order] = (i % CORES) * R + (i // CORES)

    ndst = pos[dst]
    nsrc = pos[src]
    idxq_all = (nsrc >> 1).astype(np.int16)
    m_all = (nsrc & 1).astype(np.float32)

    # per-block widths D_j (max over cores); deal => profile ~identical
    degq = np.zeros(NPAD, np.int64)
    degq[pos] = deg
    D = degq.reshape(CORES, NB_C, P).max(axis=(0, 2))
    D = np.maximum(D, 1)

    assert D.max() <= CH_COLS, f"max run width {D.max()} exceeds {CH_COLS}"
    groups = []  # (j0, nblk, G)
    j = 0
    while j < NB_C:
        j0 = j
        g = 0
        while j < NB_C:
            if j > j0 and max(g, int(D[j])) * (j - j0 + 1) > CH_COLS:
                break
            g = max(g, int(D[j]))
            j += 1
        groups.append((j0, j - j0, g))
    gspec = tuple(groups)
    total_cols = sum(nb * g for (_, nb, g) in gspec)

    inv_deg_new = np.ones(NPAD, np.float32)
    vmask = np.zeros(NPAD, bool)
    vmask[pos] = True
    inv_deg_new[vmask] = (1.0 / np.maximum(degq[vmask], 1)).astype(np.float32)
    xp = np.zeros((NPAD, IN_F), np.float32)
    xp[pos] = x

    percore = []
    for c in range(CORES):
        sel = (ndst >= c * R) & (ndst < (c + 1) * R)
        r_e = ndst[sel] - c * R
        q_e = idxq_all[sel]
        pm_e = m_all[sel]
        o = np.argsort(r_e, kind="stable")
        r_s = r_e[o]
        q_s = q_e[o]
        m_s = pm_e[o]
        cnt = np.bincount(r_s, minlength=R)
        st = np.zeros(R + 1, np.int64)
        np.cumsum(cnt, out=st[1:])
        t_s = np.arange(r_s.size) - st[r_s]

        jj_s = r_s // P
        pp_s = r_s % P

        idx_stream = np.full(total_cols * P, ZQ, np.int16)
        m_arr = np.zeros((P, total_cols), np.float32)
        colbase = 0
        for (j0, nblk, G) in gspec:
            msel = (jj_s >= j0) & (jj_s < j0 + nblk)
            col = colbase + (jj_s[msel] - j0) * G + t_s[msel]
            idx_stream[col * P + pp_s[msel]] = q_s[msel]
            m_arr[pp_s[msel], col] = m_s[msel]
            colbase += nblk * G
        n_all = total_cols * P
        wrapped = np.zeros((16, n_all // 16), np.int16)
        wrapped[np.arange(n_all) % 16, np.arange(n_all) // 16] = idx_stream
        idx_t = np.tile(wrapped, (8, 1))

        iv = inv_deg_new[c * R : (c + 1) * R]
        invd_exp = np.repeat(iv.reshape(NB_C, P).T[:, :, None], HID_F,
                             axis=2).reshape(P, W)
        xT = xp[c * R : (c + 1) * R].T.astype(np.float32).copy()
        percore.append((xT, idx_t, m_arr.astype(np.float32),
                        np.ascontiguousarray(invd_exp)))
    return gspec, pos, percore


def _build(gspec, has_b1, has_b2):
    """Build + compile the SPMD bass program (uniform across cores)."""
    import concourse.bacc as bacc
    import concourse.bass as bass
    import concourse.mybir as mybir
    import concourse.tile as tile
    from concourse import library_config

    f32 = mybir.dt.float32
    bf16 = mybir.dt.bfloat16
    i16 = mybir.dt.int16
    TOTC = sum(nb * g for (_, nb, g) in gspec)
    IDXC = TOTC * P // 16

    nc = bacc.Bacc("TRN2", target_bir_lowering=False, debug=False,
                   num_devices=CORES, num_swdge_queues=4)

    xT_d = nc.dram_tensor("xT", [P, R], f32, kind="ExternalInput")
    idx_d = nc.dram_tensor("idxs", [P, IDXC], i16, kind="ExternalInput")
    m_d = nc.dram_tensor("mpar", [P, TOTC], f32, kind="ExternalInput")
    invd_d = nc.dram_tensor("invd", [P, W], f32, kind="ExternalInput")
    w1n_d = nc.dram_tensor("w1n", [IN_F, HID_F], f32, kind="ExternalInput")
    w1s_d = nc.dram_tensor("w1s", [IN_F, HID_F], f32, kind="ExternalInput")
    w2n_d = nc.dram_tensor("w2n", [HID_F, OUT_F], f32, kind="ExternalInput")
    w2s_d = nc.dram_tensor("w2s", [HID_F, OUT_F], f32, kind="ExternalInput")
    b1_d = nc.dram_tensor("b1r", [P, HID_F], f32, kind="ExternalInput")
    b2_d = nc.dram_tensor("b2r", [P, OUT_F], f32, kind="ExternalInput")
    out_d = nc.dram_tensor("out", [R, OUT_F], f32, kind="ExternalOutput")

    cc_in1 = nc.dram_tensor("cc_in1", [R, HID_F], bf16)
    cc_out1 = nc.dram_tensor("cc_out1", [NPAD, HID_F], bf16)
    cc_in2 = nc.dram_tensor("cc_in2", [R, HID_F], bf16)
    cc_out2 = nc.dram_tensor("cc_out2", [NPAD, HID_F], bf16)

    groups_rg = [list(range(CORES))]
    mul = mybir.AluOpType.mult
    sub = mybir.AluOpType.subtract
    relu = mybir.ActivationFunctionType.Relu

    from concourse.masks import make_identity

    with tile.TileContext(nc) as tc:
        with (
            tc.tile_pool(name="pers", bufs=1) as pers,
            tc.tile_pool(name="stage", bufs=4) as stage,
            tc.tile_pool(name="graw", bufs=2) as graw_pool,
            tc.tile_pool(name="gsel", bufs=2) as gsel_pool,
            tc.tile_pool(name="pproj", bufs=2, space="PSUM") as pproj,
            tc.tile_pool(name="pself", bufs=2, space="PSUM") as pself,
            tc.tile_pool(name="ptr", bufs=2, space="PSUM") as ptr_pool,
        ):
            nc.gpsimd.load_library(library_config.mlp)
            xT = pers.tile([P, R], f32)
            nc.sync.dma_start(out=xT[:], in_=xT_d[:, :])
            idxs = pers.tile([P, IDXC], i16)
            nc.sync.dma_start(out=idxs[:], in_=idx_d[:, :])
            mpar = pers.tile([P, TOTC], f32)
            nc.sync.dma_start(out=mpar[:], in_=m_d[:, :])
            invd = pers.tile([P, W], f32)
            nc.sync.dma_start(out=invd[:], in_=invd_d[:, :])
            w1n = pers.tile([IN_F, HID_F], f32)
            nc.sync.dma_start(out=w1n[:], in_=w1n_d[:, :])
            w1s = pers.tile([IN_F, HID_F], f32)
            nc.sync.dma_start(out=w1s[:], in_=w1s_d[:, :])
            w2n = pers.tile([HID_F, OUT_F], f32)
            nc.sync.dma_start(out=w2n[:], in_=w2n_d[:, :])
            w2s = pers.tile([HID_F, OUT_F], f32)
            nc.sync.dma_start(out=w2s[:], in_=w2s_d[:, :])
            b1r = pers.tile([P, HID_F], f32)
            if has_b1:
                nc.sync.dma_start(out=b1r[:], in_=b1_d[:, :])
            b2r = pers.tile([P, OUT_F], f32)
            if has_b2:
                nc.sync.dma_start(out=b2r[:], in_=b2_d[:, :])
            ident = pers.tile([P, P], f32)
            make_identity(nc, ident[:])
            h1 = pers.tile([P, W], f32)
            h1T = pers.tile([HID_F, R], f32)
            agg = pers.tile([P, W], f32)
            hs = pers.tile([P, W], f32)

            def proj_blocks(lhsT_of, w, cc_in):
                for b in range(NB_C):
                    ps = pproj.tile([P, HID_F], f32, tag="proj")
                    nc.tensor.matmul(out=ps[:], lhsT=lhsT_of(b), rhs=w[:],
                                     start=True, stop=True)
                    t = stage.tile([P, HID_F], bf16, tag="proj_sb")
                    nc.vector.tensor_copy(out=t[:], in_=ps[:])
                    nc.sync.dma_start(out=cc_in[b * P : (b + 1) * P, :], in_=t[:])

            def self_blocks(lhsT_of, w):
                for b in range(NB_C):
                    ps = pself.tile([P, HID_F], f32, tag="self")
                    nc.tensor.matmul(out=ps[:], lhsT=lhsT_of(b), rhs=w[:],
                                     start=True, stop=True)
                    nc.vector.tensor_copy(out=hs[:, b * HID_F : (b + 1) * HID_F],
                                          in_=ps[:])

            def agg_passes(cc_out):
                ccv = cc_out[:, :].rearrange("(q two) f -> q (two f)", two=2)
                colbase = 0
                qn = 0
                for (j0, nblk, G) in gspec:
                    ncols = nblk * G
                    n_tok = ncols * P
                    g = graw_pool.tile([P, CH_COLS, 2 * HID_F], bf16, tag="g")
                    nc.gpsimd.dma_gather(
                        g[:, 0:ncols, :], ccv,
                        idxs[:, colbase * P // 16 : (colbase + ncols) * P // 16],
                        n_tok, n_tok, 2 * HID_F, single_packet=False,
                        queue_num=qn)
                    qn = (qn + 1) % 4
                    lo = g[:, 0:ncols, 0:HID_F]
                    hi = g[:, 0:ncols, HID_F : 2 * HID_F]
                    mB = mpar[:, colbase : colbase + ncols].unsqueeze(2) \
                        .to_broadcast([P, ncols, HID_F])
                    s = gsel_pool.tile([P, CH_COLS, HID_F], bf16, tag="s")
                    sc = s[:, 0:ncols, :]
                    sv = sc.rearrange("p (nb gw) f -> p nb gw f",
                                      nb=nblk, gw=G)
                    nc.vector.tensor_tensor(out=sc, in0=hi, in1=lo, op=sub)
                    nc.vector.tensor_tensor(out=sc, in0=sc, in1=mB, op=mul)
                    nc.vector.tensor_add(out=sc, in0=sc, in1=lo)
                    aggv = agg[:, j0 * HID_F : (j0 + nblk) * HID_F].rearrange(
                        "p (nb f) -> p nb f", nb=nblk)
                    D = G
                    while D > 2:
                        h = D // 2
                        nc.vector.tensor_add(
                            out=sv[:, :, 0:h, :], in0=sv[:, :, 0:h, :],
                            in1=sv[:, :, D - h : D, :])
                        D = D - h
                    if D == 2:
                        nc.vector.tensor_add(out=aggv, in0=sv[:, :, 0, :],
                                             in1=sv[:, :, 1, :])
                    else:
                        nc.vector.tensor_copy(out=aggv, in_=sv[:, :, 0, :])
                    colbase += ncols

            def combine(has_b, br, out_tile):
                nc.vector.tensor_tensor(out=agg[:], in0=agg[:], in1=invd[:],
                                        op=mul)
                nc.vector.tensor_add(out=agg[:], in0=agg[:], in1=hs[:])
                if has_b:
                    for b in range(NB_C):
                        nc.vector.tensor_add(
                            out=agg[:, b * HID_F : (b + 1) * HID_F],
                            in0=agg[:, b * HID_F : (b + 1) * HID_F], in1=br[:])
                nc.scalar.activation(out=out_tile[:], in_=agg[:], func=relu)

            NPADROWS = R - NPC  # 22 pad rows: block 48, partitions 106..127

            # ---- layer 1
            proj_blocks(lambda b: xT[:, b * P : (b + 1) * P], w1n, cc_in1)
            nc.gpsimd.collective_compute(
                "AllGather", mybir.AluOpType.bypass, replica_groups=groups_rg,
                ins=[cc_in1.ap().opt()], outs=[cc_out1.ap().opt()])
            self_blocks(lambda b: xT[:, b * P : (b + 1) * P], w1s)
            agg_passes(cc_out1)
            combine(has_b1, b1r, h1)
            if has_b1:
                # keep pad rows zero so the zero pair-row stays zero in layer 2
                nc.vector.memset(
                    h1[P - NPADROWS : P, (NB_C - 1) * HID_F : NB_C * HID_F], 0.0)

            # h1 -> h1T (PE transpose) + proj2 -> cc_in2
            for b in range(NB_C):
                pt = ptr_pool.tile([HID_F, P], f32, tag="tr")
                nc.tensor.transpose(out=pt[:],
                                    in_=h1[:, b * HID_F : (b + 1) * HID_F],
                                    identity=ident[:])
                nc.vector.tensor_copy(out=h1T[:, b * P : (b + 1) * P], in_=pt[:])
                ps = pproj.tile([P, HID_F], f32, tag="proj")
                nc.tensor.matmul(out=ps[:], lhsT=h1T[:, b * P : (b + 1) * P],
                                 rhs=w2n[:], start=True, stop=True)
                t = stage.tile([P, HID_F], bf16, tag="proj_sb")
                nc.vector.tensor_copy(out=t[:], in_=ps[:])
                nc.sync.dma_start(out=cc_in2[b * P : (b + 1) * P, :], in_=t[:])

            # ---- layer 2
            nc.gpsimd.collective_compute(
                "AllGather", mybir.AluOpType.bypass, replica_groups=groups_rg,
                ins=[cc_in2.ap().opt()], outs=[cc_out2.ap().opt()])
            self_blocks(lambda b: h1T[:, b * P : (b + 1) * P], w2s)
            agg_passes(cc_out2)
            combine(has_b2, b2r, hs)
            for b in range(NB_C):
                nc.sync.dma_start(out=out_d[b * P : (b + 1) * P, :],
                                  in_=hs[:, b * HID_F : (b + 1) * HID_F])

    nc.compile()
    return nc


def _run(inputs, trace=False, tmpdir=None):
    from concourse.bass_utils import run_bass_kernel_spmd

    x = np.asarray(inputs["x"], np.float32)
    src = np.asarray(inputs["src"])
    dst = np.asarray(inputs["dst"])
    gspec, pos, percore = _prep(x, src, dst)
    b1 = np.asarray(inputs["b1"], np.float32)
    b2 = np.asarray(inputs["b2"], np.float32)
    has_b1 = bool(np.any(b1))
    has_b2 = bool(np.any(b2))

    key = (gspec, has_b1, has_b2)
    if key not in _cache:
        _cache[key] = _build(gspec, has_b1, has_b2)
    nc = _cache[key]

    shared = {
        "w1n": np.asarray(inputs["W1_neigh"], np.float32),
        "w1s": np.asarray(inputs["W1_self"], np.float32),
        "w2n": np.asarray(inputs["W2_neigh"], np.float32),
        "w2s": np.asarray(inputs["W2_self"], np.float32),
        "b1r": np.broadcast_to(b1, (P, HID_F)).copy(),
        "b2r": np.broadcast_to(b2, (P, OUT_F)).copy(),
    }
    in_maps = []
    for c in range(CORES):
        xT, idx_t, m_arr, invd_exp = percore[c]
        mp = dict(shared)
        mp.update({"xT": xT, "idxs": idx_t, "mpar": m_arr, "invd": invd_exp})
        in_maps.append(mp)

    res = run_bass_kernel_spmd(nc, in_maps, list(range(CORES)),
                               trace=trace, tmpdir=tmpdir)
    h2_new = np.concatenate([res.results[c]["out"] for c in range(CORES)], axis=0)
    out = h2_new[pos]
    return out.astype(np.float32), res


def kernel(**inputs) -> np.ndarray:
    out, _ = _run(inputs, trace=False)
    return out


# revision 22
# speedup vs baseline: 1.2864x; 1.2864x over previous
"""GraphSAGE 2-layer kernel for 8 Trainium2 NeuronCores.

Strategy (dst-partitioned, batched ucode gather + DVE select/fold reduction):
  - Nodes dealt round-robin by global in-degree rank to the 8 cores, so every
    core's rank-r row has ~the same degree -> per-block run widths are
    near-uniform and padding is small (~5%).
  - Pre-projection before the gather: p = h @ W_neigh per-core, AllGather'd
    to a bf16 [NPAD, 64] table in DRAM. The gather reads it through a
    [NPAD/2, 128] pair view: one 256B gather row = nodes (2q, 2q+1), so the
    int16 gather index q = src>>1 covers the whole table in one window.
  - Aggregation per block-group: one dma_gather pulls all edge tokens
    (token i -> partition i%128, column i//128; layout [128, nblk, G, 128]),
    a 3-op DVE select picks the src&1 half per token (f32 out), and a
    log-depth in-place strided DVE fold sums each row's G-token run, with the
    last step written straight into the agg columns. Pad tokens point at an
    all-zero pair row.
  - h = relu(x @ W_self + inv_deg * agg + b) with full-width in-place DVE ops
    and one scalar-engine relu; layer 2 identical (same graph -> same index
    streams) with an on-chip PE transpose of h1.
"""

import numpy as np

N = 50000
E = 800000
IN_F, HID_F, OUT_F = 128, 64, 64
CORES = 8
P = 128
NPC = N // CORES          # 6250 real nodes per core
NB_C = 49                 # dst blocks per core
R = NB_C * P              # 6272 padded rows per core
NPAD = CORES * R          # 50176
W = NB_C * HID_F          # 3136 full-width columns
ZQ = (NPC) // 2           # 3125: pair (6250, 6251) = core 0 pad rows, all-zero
CH_COLS = 56              # max token-columns per gather (7168 tokens)

_cache = {}


def _prep(x, src, dst):
    """Host-side: degree-deal nodes to cores, build grouped token streams."""
    src = np.asarray(src).astype(np.int64)
    dst = np.asarray(dst).astype(np.int64)
    deg = np.bincount(dst, minlength=N)

    order = np.argsort(-deg, kind="stable")
    pos = np.empty(N, np.int64)
    i = np.arange(N)
    pos[order] = (i % CORES) * R + (i // CORES)

    ndst = pos[dst]
    nsrc = pos[src]
    idxq_all = (nsrc >> 1).astype(np.int16)
    m_all = (nsrc & 1).astype(np.float32)

    # per-block widths D_j (max over cores); deal => profile ~identical
    degq = np.zeros(NPAD, np.int64)
    degq[pos] = deg
    D = degq.reshape(CORES, NB_C, P).max(axis=(0, 2))
    D = np.maximum(D, 1)

    assert D.max() <= CH_COLS, f"max run width {D.max()} exceeds {CH_COLS}"
    groups = []  # (j0, nblk, G)
    j = 0
    while j < NB_C:
        j0 = j
        g = 0
        while j < NB_C:
            if j > j0 and max(g, int(D[j])) * (j - j0 + 1) > CH_COLS:
                break
            g = max(g, int(D[j]))
            j += 1
        groups.append((j0, j - j0, g))
    gspec = tuple(groups)
    total_cols = sum(nb * g for (_, nb, g) in gspec)

    inv_deg_new = np.ones(NPAD, np.float32)
    vmask = np.zeros(NPAD, bool)
    vmask[pos] = True
    inv_deg_new[vmask] = (1.0 / np.maximum(degq[vmask], 1)).astype(np.float32)
    xp = np.zeros((NPAD, IN_F), np.float32)
    xp[pos] = x

    percore = []
    for c in range(CORES):
        sel = (ndst >= c * R) & (ndst < (c + 1) * R)
        r_e = ndst[sel] - c * R
        q_e = idxq_all[sel]
        pm_e = m_all[sel]
        o = np.argsort(r_e, kind="stable")
        r_s = r_e[o]
        q_s = q_e[o]
        m_s = pm_e[o]
        cnt = np.bincount(r_s, minlength=R)
        st = np.zeros(R + 1, np.int64)
        np.cumsum(cnt, out=st[1:])
        t_s = np.arange(r_s.size) - st[r_s]

        jj_s = r_s // P
        pp_s = r_s % P

        idx_stream = np.full(total_cols * P, ZQ, np.int16)
        m_arr = np.zeros((P, total_cols), np.float32)
        colbase = 0
        for (j0, nblk, G) in gspec:
            msel = (jj_s >= j0) & (jj_s < j0 + nblk)
            col = colbase + (jj_s[msel] - j0) * G + t_s[msel]
            idx_stream[col * P + pp_s[msel]] = q_s[msel]
            m_arr[pp_s[msel], col] = m_s[msel]
            colbase += nblk * G
        n_all = total_cols * P
        wrapped = np.zeros((16, n_all // 16), np.int16)
        wrapped[np.arange(n_all) % 16, np.arange(n_all) // 16] = idx_stream
        idx_t = np.tile(wrapped, (8, 1))

        iv = inv_deg_new[c * R : (c + 1) * R]
        invd_exp = np.repeat(iv.reshape(NB_C, P).T[:, :, None], HID_F,
                             axis=2).reshape(P, W)
        xT = xp[c * R : (c + 1) * R].T.astype(np.float32).copy()
        percore.append((xT, idx_t, m_arr.astype(np.float32),
                        np.ascontiguousarray(invd_exp)))
    return gspec, pos, percore


def _build(gspec, has_b1, has_b2):
    """Build + compile the SPMD bass program (uniform across cores)."""
    import concourse.bacc as bacc
    import concourse.bass as bass
    import concourse.mybir as mybir
    import concourse.tile as tile
    from concourse import library_config

    f32 = mybir.dt.float32
    bf16 = mybir.dt.bfloat16
    i16 = mybir.dt.int16
    TOTC = sum(nb * g for (_, nb, g) in gspec)
    IDXC = TOTC * P // 16

    nc = bacc.Bacc("TRN2", target_bir_lowering=False, debug=False,
                   num_devices=CORES, num_swdge_queues=4)

    xT_d = nc.dram_tensor("xT", [P, R], f32, kind="ExternalInput")
    idx_d = nc.dram_tensor("idxs", [P, IDXC], i16, kind="ExternalInput")
    m_d = nc.dram_tensor("mpar", [P, TOTC], f32, kind="ExternalInput")
    invd_d = nc.dram_tensor("invd", [P, W], f32, kind="ExternalInput")
    w1n_d = nc.dram_tensor("w1n", [IN_F, HID_F], f32, kind="ExternalInput")
    w1s_d = nc.dram_tensor("w1s", [IN_F, HID_F], f32, kind="ExternalInput")
    w2n_d = nc.dram_tensor("w2n", [HID_F, OUT_F], f32, kind="ExternalInput")
    w2s_d = nc.dram_tensor("w2s", [HID_F, OUT_F], f32, kind="ExternalInput")
    b1_d = nc.dram_tensor("b1r", [P, HID_F], f32, kind="ExternalInput")
    b2_d = nc.dram_tensor("b2r", [P, OUT_F], f32, kind="ExternalInput")
    out_d = nc.dram_tensor("out", [R, OUT_F], f32, kind="ExternalOutput")

    cc_in1 = nc.dram_tensor("cc_in1", [R, HID_F], bf16)
    cc_out1 = nc.dram_tensor("cc_out1", [NPAD, HID_F], bf16)
    cc_in2 = nc.dram_tensor("cc_in2", [R, HID_F], bf16)
    cc_out2 = nc.dram_tensor("cc_out2", [NPAD, HID_F], bf16)

    groups_rg = [list(range(CORES))]
    mul = mybir.AluOpType.mult
    sub = mybir.AluOpType.subtract
    relu = mybir.ActivationFunctionType.Relu

    from concourse.masks import make_identity

    with tile.TileContext(nc) as tc:
        with (
            tc.tile_pool(name="pers", bufs=1) as pers,
            tc.tile_pool(name="stage", bufs=4) as stage,
            tc.tile_pool(name="graw", bufs=3) as graw_pool,
            tc.tile_pool(name="gd", bufs=2) as gd_pool,
            tc.tile_pool(name="gsel", bufs=2) as gsel_pool,
            tc.tile_pool(name="pproj", bufs=2, space="PSUM") as pproj,
            tc.tile_pool(name="pself", bufs=2, space="PSUM") as pself,
            tc.tile_pool(name="ptr", bufs=2, space="PSUM") as ptr_pool,
        ):
            nc.gpsimd.load_library(library_config.mlp)
            xT = pers.tile([P, R], f32)
            nc.sync.dma_start(out=xT[:], in_=xT_d[:, :])
            idxs = pers.tile([P, IDXC], i16)
            nc.sync.dma_start(out=idxs[:], in_=idx_d[:, :])
            mpar = pers.tile([P, TOTC], f32)
            nc.sync.dma_start(out=mpar[:], in_=m_d[:, :])
            invd = pers.tile([P, W], f32)
            nc.sync.dma_start(out=invd[:], in_=invd_d[:, :])
            w1n = pers.tile([IN_F, HID_F], f32)
            nc.sync.dma_start(out=w1n[:], in_=w1n_d[:, :])
            w1s = pers.tile([IN_F, HID_F], f32)
            nc.sync.dma_start(out=w1s[:], in_=w1s_d[:, :])
            w2n = pers.tile([HID_F, OUT_F], f32)
            nc.sync.dma_start(out=w2n[:], in_=w2n_d[:, :])
            w2s = pers.tile([HID_F, OUT_F], f32)
            nc.sync.dma_start(out=w2s[:], in_=w2s_d[:, :])
            b1r = pers.tile([P, HID_F], f32)
            if has_b1:
                nc.sync.dma_start(out=b1r[:], in_=b1_d[:, :])
            b2r = pers.tile([P, OUT_F], f32)
            if has_b2:
                nc.sync.dma_start(out=b2r[:], in_=b2_d[:, :])
            ident = pers.tile([P, P], f32)
            make_identity(nc, ident[:])
            h1 = pers.tile([P, W], f32)
            h1T = pers.tile([HID_F, R], f32)
            agg = pers.tile([P, W], f32)
            hs = pers.tile([P, W], f32)

            def proj_blocks(lhsT_of, w, cc_in):
                for b in range(NB_C):
                    ps = pproj.tile([P, HID_F], f32, tag="proj")
                    nc.tensor.matmul(out=ps[:], lhsT=lhsT_of(b), rhs=w[:],
                                     start=True, stop=True)
                    t = stage.tile([P, HID_F], bf16, tag="proj_sb")
                    nc.vector.tensor_copy(out=t[:], in_=ps[:])
                    nc.sync.dma_start(out=cc_in[b * P : (b + 1) * P, :], in_=t[:])

            def self_blocks(lhsT_of, w):
                for b in range(NB_C):
                    ps = pself.tile([P, HID_F], f32, tag="self")
                    nc.tensor.matmul(out=ps[:], lhsT=lhsT_of(b), rhs=w[:],
                                     start=True, stop=True)
                    nc.vector.tensor_copy(out=hs[:, b * HID_F : (b + 1) * HID_F],
                                          in_=ps[:])

            def agg_passes(cc_out):
                ccv = cc_out[:, :].rearrange("(q two) f -> q (two f)", two=2)
                colbase = 0
                qn = 0
                for (j0, nblk, G) in gspec:
                    ncols = nblk * G
                    n_tok = ncols * P
                    g = graw_pool.tile([P, CH_COLS, 2 * HID_F], bf16, tag="g")
                    nc.gpsimd.dma_gather(
                        g[:, 0:ncols, :], ccv,
                        idxs[:, colbase * P // 16 : (colbase + ncols) * P // 16],
                        n_tok, n_tok, 2 * HID_F, single_packet=False,
                        queue_num=qn)
                    qn = (qn + 1) % 4
                    lo = g[:, 0:ncols, 0:HID_F]
                    hi = g[:, 0:ncols, HID_F : 2 * HID_F]
                    mB = mpar[:, colbase : colbase + ncols].unsqueeze(2) \
                        .to_broadcast([P, ncols, HID_F])
                    d = gd_pool.tile([P, CH_COLS, HID_F], bf16, tag="d")
                    dv = d[:, 0:ncols, :]
                    nc.vector.tensor_tensor(out=dv, in0=hi, in1=lo, op=sub)
                    nc.vector.tensor_tensor(out=dv, in0=dv, in1=mB, op=mul)
                    s = gsel_pool.tile([P, CH_COLS, HID_F], f32, tag="s")
                    sv = s[:, 0:ncols, :].rearrange("p (nb gw) f -> p nb gw f",
                                                    nb=nblk, gw=G)
                    nc.vector.tensor_add(out=s[:, 0:ncols, :], in0=lo, in1=dv)
                    aggv = agg[:, j0 * HID_F : (j0 + nblk) * HID_F].rearrange(
                        "p (nb f) -> p nb f", nb=nblk)
                    D = G
                    while D > 2:
                        h = D // 2
                        nc.vector.tensor_add(
                            out=sv[:, :, 0:h, :], in0=sv[:, :, 0:h, :],
                            in1=sv[:, :, D - h : D, :])
                        D = D - h
                    if D == 2:
                        nc.vector.tensor_add(out=aggv, in0=sv[:, :, 0, :],
                                             in1=sv[:, :, 1, :])
                    else:
                        nc.vector.tensor_copy(out=aggv, in_=sv[:, :, 0, :])
                    colbase += ncols

            def combine(has_b, br, out_tile):
                nc.vector.tensor_tensor(out=agg[:], in0=agg[:], in1=invd[:],
                                        op=mul)
                nc.vector.tensor_add(out=agg[:], in0=agg[:], in1=hs[:])
                if has_b:
                    for b in range(NB_C):
                        nc.vector.tensor_add(
                            out=agg[:, b * HID_F : (b + 1) * HID_F],
                            in0=agg[:, b * HID_F : (b + 1) * HID_F], in1=br[:])
                nc.scalar.activation(out=out_tile[:], in_=agg[:], func=relu)

            NPADROWS = R - NPC  # 22 pad rows: block 48, partitions 106..127

            # ---- layer 1
            proj_blocks(lambda b: xT[:, b * P : (b + 1) * P], w1n, cc_in1)
            nc.gpsimd.collective_compute(
                "AllGather", mybir.AluOpType.bypass, replica_groups=groups_rg,
                ins=[cc_in1.ap().opt()], outs=[cc_out1.ap().opt()])
            self_blocks(lambda b: xT[:, b * P : (b + 1) * P], w1s)
            agg_passes(cc_out1)
            combine(has_b1, b1r, h1)
            if has_b1:
                # keep pad rows zero so the zero pair-row stays zero in layer 2
                nc.vector.memset(
                    h1[P - NPADROWS : P, (NB_C - 1) * HID_F : NB_C * HID_F], 0.0)

            # h1 -> h1T (PE transpose) + proj2 -> cc_in2
            for b in range(NB_C):
                pt = ptr_pool.tile([HID_F, P], f32, tag="tr")
                nc.tensor.transpose(out=pt[:],
                                    in_=h1[:, b * HID_F : (b + 1) * HID_F],
                                    identity=ident[:])
                nc.vector.tensor_copy(out=h1T[:, b * P : (b + 1) * P], in_=pt[:])
                ps = pproj.tile([P, HID_F], f32, tag="proj")
                nc.tensor.matmul(out=ps[:], lhsT=h1T[:, b * P : (b + 1) * P],
                                 rhs=w2n[:], start=True, stop=True)
                t = stage.tile([P, HID_F], bf16, tag="proj_sb")
                nc.vector.tensor_copy(out=t[:], in_=ps[:])
                nc.sync.dma_start(out=cc_in2[b * P : (b + 1) * P, :], in_=t[:])

            # ---- layer 2
            nc.gpsimd.collective_compute(
                "AllGather", mybir.AluOpType.bypass, replica_groups=groups_rg,
                ins=[cc_in2.ap().opt()], outs=[cc_out2.ap().opt()])
            self_blocks(lambda b: h1T[:, b * P : (b + 1) * P], w2s)
            agg_passes(cc_out2)
            combine(has_b2, b2r, hs)
            for b in range(NB_C):
                nc.sync.dma_start(out=out_d[b * P : (b + 1) * P, :],
                                  in_=hs[:, b * HID_F : (b + 1) * HID_F])

    nc.compile()
    return nc


def _run(inputs, trace=False, tmpdir=None):
    from concourse.bass_utils import run_bass_kernel_spmd

    x = np.asarray(inputs["x"], np.float32)
    src = np.asarray(inputs["src"])
    dst = np.asarray(inputs["dst"])
    gspec, pos, percore = _prep(x, src, dst)
    b1 = np.asarray(inputs["b1"], np.float32)
    b2 = np.asarray(inputs["b2"], np.float32)
    has_b1 = bool(np.any(b1))
    has_b2 = bool(np.any(b2))

    key = (gspec, has_b1, has_b2)
    if key not in _cache:
        _cache[key] = _build(gspec, has_b1, has_b2)
    nc = _cache[key]

    shared = {
        "w1n": np.asarray(inputs["W1_neigh"], np.float32),
        "w1s": np.asarray(inputs["W1_self"], np.float32),
        "w2n": np.asarray(inputs["W2_neigh"], np.float32),
        "w2s": np.asarray(inputs["W2_self"], np.float32),
        "b1r": np.broadcast_to(b1, (P, HID_F)).copy(),
        "b2r": np.broadcast_to(b2, (P, OUT_F)).copy(),
    }
    in_maps = []
    for c in range(CORES):
        xT, idx_t, m_arr, invd_exp = percore[c]
        mp = dict(shared)
        mp.update({"xT": xT, "idxs": idx_t, "mpar": m_arr, "invd": invd_exp})
        in_maps.append(mp)

    res = run_bass_kernel_spmd(nc, in_maps, list(range(CORES)),
                               trace=trace, tmpdir=tmpdir)
    h2_new = np.concatenate([res.results[c]["out"] for c in range(CORES)], axis=0)
    out = h2_new[pos]
    return out.astype(np.float32), res


def kernel(**inputs) -> np.ndarray:
    out, _ = _run(inputs, trace=False)
    return out


# revision 23
# speedup vs baseline: 1.5358x; 1.1939x over previous
"""GraphSAGE 2-layer kernel for 8 Trainium2 NeuronCores.

Strategy (dst-partitioned, batched ucode gather + DVE select/fold reduction):
  - Nodes dealt round-robin by global in-degree rank to the 8 cores, so every
    core's rank-r row has ~the same degree -> per-block run widths are
    near-uniform and padding is small (~5%).
  - Pre-projection before the gather: p = h @ W_neigh per-core, AllGather'd
    to a bf16 [NPAD, 64] table in DRAM. The gather reads it through a
    [NPAD/2, 128] pair view: one 256B gather row = nodes (2q, 2q+1), so the
    int16 gather index q = src>>1 covers the whole table in one window.
  - Aggregation per block-group: one dma_gather pulls all edge tokens
    (token i -> partition i%128, column i//128; layout [128, nblk, G, 128]),
    a 3-op DVE select picks the src&1 half per token (f32 out), and a
    log-depth in-place strided DVE fold sums each row's G-token run, with the
    last step written straight into the agg columns. Pad tokens point at an
    all-zero pair row.
  - h = relu(x @ W_self + inv_deg * agg + b) with full-width in-place DVE ops
    and one scalar-engine relu; layer 2 identical (same graph -> same index
    streams) with an on-chip PE transpose of h1.
"""

import numpy as np

N = 50000
E = 800000
IN_F, HID_F, OUT_F = 128, 64, 64
CORES = 8
P = 128
NPC = N // CORES          # 6250 real nodes per core
NB_C = 49                 # dst blocks per core
R = NB_C * P              # 6272 padded rows per core
NPAD = CORES * R          # 50176
W = NB_C * HID_F          # 3136 full-width columns
ZQ = (NPC) // 2           # 3125: pair (6250, 6251) = core 0 pad rows, all-zero
CH_COLS = 56              # max token-columns per gather (7168 tokens)

_cache = {}


def _prep(x, src, dst):
    """Host-side: degree-deal nodes to cores, build grouped token streams."""
    src = np.asarray(src).astype(np.int64)
    dst = np.asarray(dst).astype(np.int64)
    deg = np.bincount(dst, minlength=N)

    order = np.argsort(-deg, kind="stable")
    pos = np.empty(N, np.int64)
    i = np.arange(N)
    pos[order] = (i % CORES) * R + (i // CORES)

    ndst = pos[dst]
    nsrc = pos[src]
    idxq_all = (nsrc >> 1).astype(np.int16)
    m_all = (nsrc & 1).astype(np.float32)

    # per-block widths D_j (max over cores); deal => profile ~identical
    degq = np.zeros(NPAD, np.int64)
    degq[pos] = deg
    D = degq.reshape(CORES, NB_C, P).max(axis=(0, 2))
    D = np.maximum(D, 1)

    assert D.max() <= CH_COLS, f"max run width {D.max()} exceeds {CH_COLS}"
    groups = []  # (j0, nblk, G)
    j = 0
    while j < NB_C:
        j0 = j
        g = 0
        while j < NB_C:
            if j > j0 and max(g, int(D[j])) * (j - j0 + 1) > CH_COLS:
                break
            g = max(g, int(D[j]))
            j += 1
        groups.append((j0, j - j0, g))
    gspec = tuple(groups)
    total_cols = sum(nb * g for (_, nb, g) in gspec)

    inv_deg_new = np.ones(NPAD, np.float32)
    vmask = np.zeros(NPAD, bool)
    vmask[pos] = True
    inv_deg_new[vmask] = (1.0 / np.maximum(degq[vmask], 1)).astype(np.float32)
    xp = np.zeros((NPAD, IN_F), np.float32)
    xp[pos] = x

    percore = []
    for c in range(CORES):
        sel = (ndst >= c * R) & (ndst < (c + 1) * R)
        r_e = ndst[sel] - c * R
        q_e = idxq_all[sel]
        pm_e = m_all[sel]
        o = np.argsort(r_e, kind="stable")
        r_s = r_e[o]
        q_s = q_e[o]
        m_s = pm_e[o]
        cnt = np.bincount(r_s, minlength=R)
        st = np.zeros(R + 1, np.int64)
        np.cumsum(cnt, out=st[1:])
        t_s = np.arange(r_s.size) - st[r_s]

        jj_s = r_s // P
        pp_s = r_s % P

        idx_stream = np.full(total_cols * P, ZQ, np.int16)
        m_arr = np.zeros((P, total_cols), np.float32)
        colbase = 0
        for (j0, nblk, G) in gspec:
            msel = (jj_s >= j0) & (jj_s < j0 + nblk)
            col = colbase + (jj_s[msel] - j0) * G + t_s[msel]
            idx_stream[col * P + pp_s[msel]] = q_s[msel]
            m_arr[pp_s[msel], col] = m_s[msel]
            colbase += nblk * G
        n_all = total_cols * P
        wrapped = np.zeros((16, n_all // 16), np.int16)
        wrapped[np.arange(n_all) % 16, np.arange(n_all) // 16] = idx_stream
        idx_t = np.tile(wrapped, (8, 1))

        iv = inv_deg_new[c * R : (c + 1) * R]
        invd_exp = np.repeat(iv.reshape(NB_C, P).T[:, :, None], HID_F,
                             axis=2).reshape(P, W)
        xT = xp[c * R : (c + 1) * R].T.astype(np.float32).copy()
        percore.append((xT, idx_t, m_arr.astype(np.float32),
                        np.ascontiguousarray(invd_exp)))
    return gspec, pos, percore


def _build(gspec, has_b1, has_b2):
    """Build + compile the SPMD bass program (uniform across cores)."""
    import concourse.bacc as bacc
    import concourse.bass as bass
    import concourse.mybir as mybir
    import concourse.tile as tile
    from concourse import library_config

    f32 = mybir.dt.float32
    bf16 = mybir.dt.bfloat16
    i16 = mybir.dt.int16
    TOTC = sum(nb * g for (_, nb, g) in gspec)
    IDXC = TOTC * P // 16

    nc = bacc.Bacc("TRN2", target_bir_lowering=False, debug=False,
                   num_devices=CORES, num_swdge_queues=4)

    xT_d = nc.dram_tensor("xT", [P, R], f32, kind="ExternalInput")
    idx_d = nc.dram_tensor("idxs", [P, IDXC], i16, kind="ExternalInput")
    m_d = nc.dram_tensor("mpar", [P, TOTC], f32, kind="ExternalInput")
    invd_d = nc.dram_tensor("invd", [P, W], f32, kind="ExternalInput")
    w1n_d = nc.dram_tensor("w1n", [IN_F, HID_F], f32, kind="ExternalInput")
    w1s_d = nc.dram_tensor("w1s", [IN_F, HID_F], f32, kind="ExternalInput")
    w2n_d = nc.dram_tensor("w2n", [HID_F, OUT_F], f32, kind="ExternalInput")
    w2s_d = nc.dram_tensor("w2s", [HID_F, OUT_F], f32, kind="ExternalInput")
    b1_d = nc.dram_tensor("b1r", [P, HID_F], f32, kind="ExternalInput")
    b2_d = nc.dram_tensor("b2r", [P, OUT_F], f32, kind="ExternalInput")
    out_d = nc.dram_tensor("out", [R, OUT_F], f32, kind="ExternalOutput")

    cc_in1 = nc.dram_tensor("cc_in1", [R, HID_F], bf16)
    cc_out1 = nc.dram_tensor("cc_out1", [NPAD, HID_F], bf16)
    cc_in2 = nc.dram_tensor("cc_in2", [R, HID_F], bf16)
    cc_out2 = nc.dram_tensor("cc_out2", [NPAD, HID_F], bf16)

    groups_rg = [list(range(CORES))]
    mul = mybir.AluOpType.mult
    sub = mybir.AluOpType.subtract
    relu = mybir.ActivationFunctionType.Relu

    from concourse.masks import make_identity

    with tile.TileContext(nc) as tc:
        with (
            tc.tile_pool(name="pers", bufs=1) as pers,
            tc.tile_pool(name="stage", bufs=4) as stage,
            tc.tile_pool(name="graw", bufs=4) as graw_pool,
            tc.tile_pool(name="gsel", bufs=3) as gsel_pool,
            tc.tile_pool(name="pproj", bufs=2, space="PSUM") as pproj,
            tc.tile_pool(name="pself", bufs=2, space="PSUM") as pself,
            tc.tile_pool(name="ptr", bufs=2, space="PSUM") as ptr_pool,
        ):
            nc.gpsimd.load_library(library_config.mlp)
            xT = pers.tile([P, R], f32)
            nc.sync.dma_start(out=xT[:], in_=xT_d[:, :])
            idxs = pers.tile([P, IDXC], i16)
            nc.sync.dma_start(out=idxs[:], in_=idx_d[:, :])
            mpar = pers.tile([P, TOTC], f32)
            nc.sync.dma_start(out=mpar[:], in_=m_d[:, :])
            invd = pers.tile([P, W], f32)
            nc.sync.dma_start(out=invd[:], in_=invd_d[:, :])
            w1n = pers.tile([IN_F, HID_F], f32)
            nc.sync.dma_start(out=w1n[:], in_=w1n_d[:, :])
            w1s = pers.tile([IN_F, HID_F], f32)
            nc.sync.dma_start(out=w1s[:], in_=w1s_d[:, :])
            w2n = pers.tile([HID_F, OUT_F], f32)
            nc.sync.dma_start(out=w2n[:], in_=w2n_d[:, :])
            w2s = pers.tile([HID_F, OUT_F], f32)
            nc.sync.dma_start(out=w2s[:], in_=w2s_d[:, :])
            b1r = pers.tile([P, HID_F], f32)
            if has_b1:
                nc.sync.dma_start(out=b1r[:], in_=b1_d[:, :])
            b2r = pers.tile([P, OUT_F], f32)
            if has_b2:
                nc.sync.dma_start(out=b2r[:], in_=b2_d[:, :])
            ident = pers.tile([P, P], f32)
            make_identity(nc, ident[:])
            h1 = pers.tile([P, W], f32)
            h1T = pers.tile([HID_F, R], f32)
            agg = pers.tile([P, W], f32)
            hs = pers.tile([P, W], f32)

            def proj_blocks(lhsT_of, w, cc_in):
                for b in range(NB_C):
                    ps = pproj.tile([P, HID_F], f32, tag="proj")
                    nc.tensor.matmul(out=ps[:], lhsT=lhsT_of(b), rhs=w[:],
                                     start=True, stop=True)
                    t = stage.tile([P, HID_F], bf16, tag="proj_sb")
                    nc.vector.tensor_copy(out=t[:], in_=ps[:])
                    nc.sync.dma_start(out=cc_in[b * P : (b + 1) * P, :], in_=t[:])

            def self_blocks(lhsT_of, w):
                for b in range(NB_C):
                    ps = pself.tile([P, HID_F], f32, tag="self")
                    nc.tensor.matmul(out=ps[:], lhsT=lhsT_of(b), rhs=w[:],
                                     start=True, stop=True)
                    nc.vector.tensor_copy(out=hs[:, b * HID_F : (b + 1) * HID_F],
                                          in_=ps[:])

            def agg_passes(cc_out):
                ccv = cc_out[:, :].rearrange("(q two) f -> q (two f)", two=2)
                colbase = 0
                qn = 0
                for (j0, nblk, G) in gspec:
                    ncols = nblk * G
                    n_tok = ncols * P
                    g = graw_pool.tile([P, CH_COLS, 2 * HID_F], bf16, tag="g")
                    nc.gpsimd.dma_gather(
                        g[:, 0:ncols, :], ccv,
                        idxs[:, colbase * P // 16 : (colbase + ncols) * P // 16],
                        n_tok, n_tok, 2 * HID_F, single_packet=False,
                        queue_num=qn)
                    qn = (qn + 1) % 4
                    lo = g[:, 0:ncols, 0:HID_F]
                    hi = g[:, 0:ncols, HID_F : 2 * HID_F]
                    mB = mpar[:, colbase : colbase + ncols].unsqueeze(2) \
                        .to_broadcast([P, ncols, HID_F])
                    s = gsel_pool.tile([P, CH_COLS, HID_F], bf16, tag="s")
                    sc = s[:, 0:ncols, :]
                    sv = sc.rearrange("p (nb gw) f -> p nb gw f",
                                      nb=nblk, gw=G)
                    nc.vector.tensor_tensor(out=sc, in0=hi, in1=lo, op=sub)
                    nc.vector.tensor_tensor(out=sc, in0=sc, in1=mB, op=mul)
                    nc.vector.tensor_add(out=sc, in0=sc, in1=lo)
                    aggv = agg[:, j0 * HID_F : (j0 + nblk) * HID_F].rearrange(
                        "p (nb f) -> p nb f", nb=nblk)
                    D = G
                    while D > 2:
                        h = D // 2
                        nc.vector.tensor_add(
                            out=sv[:, :, 0:h, :], in0=sv[:, :, 0:h, :],
                            in1=sv[:, :, D - h : D, :])
                        D = D - h
                    if D == 2:
                        nc.vector.tensor_add(out=aggv, in0=sv[:, :, 0, :],
                                             in1=sv[:, :, 1, :])
                    else:
                        nc.vector.tensor_copy(out=aggv, in_=sv[:, :, 0, :])
                    colbase += ncols

            def combine(has_b, br, out_tile):
                nc.vector.tensor_tensor(out=agg[:], in0=agg[:], in1=invd[:],
                                        op=mul)
                nc.vector.tensor_add(out=agg[:], in0=agg[:], in1=hs[:])
                if has_b:
                    for b in range(NB_C):
                        nc.vector.tensor_add(
                            out=agg[:, b * HID_F : (b + 1) * HID_F],
                            in0=agg[:, b * HID_F : (b + 1) * HID_F], in1=br[:])
                nc.scalar.activation(out=out_tile[:], in_=agg[:], func=relu)

            NPADROWS = R - NPC  # 22 pad rows: block 48, partitions 106..127

            # ---- layer 1
            proj_blocks(lambda b: xT[:, b * P : (b + 1) * P], w1n, cc_in1)
            nc.gpsimd.collective_compute(
                "AllGather", mybir.AluOpType.bypass, replica_groups=groups_rg,
                ins=[cc_in1.ap().opt()], outs=[cc_out1.ap().opt()])
            self_blocks(lambda b: xT[:, b * P : (b + 1) * P], w1s)
            agg_passes(cc_out1)
            combine(has_b1, b1r, h1)
            if has_b1:
                # keep pad rows zero so the zero pair-row stays zero in layer 2
                nc.vector.memset(
                    h1[P - NPADROWS : P, (NB_C - 1) * HID_F : NB_C * HID_F], 0.0)

            # h1 -> h1T (PE transpose) + proj2 -> cc_in2
            for b in range(NB_C):
                pt = ptr_pool.tile([HID_F, P], f32, tag="tr")
                nc.tensor.transpose(out=pt[:],
                                    in_=h1[:, b * HID_F : (b + 1) * HID_F],
                                    identity=ident[:])
                nc.vector.tensor_copy(out=h1T[:, b * P : (b + 1) * P], in_=pt[:])
                ps = pproj.tile([P, HID_F], f32, tag="proj")
                nc.tensor.matmul(out=ps[:], lhsT=h1T[:, b * P : (b + 1) * P],
                                 rhs=w2n[:], start=True, stop=True)
                t = stage.tile([P, HID_F], bf16, tag="proj_sb")
                nc.vector.tensor_copy(out=t[:], in_=ps[:])
                nc.sync.dma_start(out=cc_in2[b * P : (b + 1) * P, :], in_=t[:])

            # ---- layer 2
            nc.gpsimd.collective_compute(
                "AllGather", mybir.AluOpType.bypass, replica_groups=groups_rg,
                ins=[cc_in2.ap().opt()], outs=[cc_out2.ap().opt()])
            self_blocks(lambda b: h1T[:, b * P : (b + 1) * P], w2s)
            agg_passes(cc_out2)
            combine(has_b2, b2r, hs)
            for b in range(NB_C):
                nc.sync.dma_start(out=out_d[b * P : (b + 1) * P, :],
                                  in_=hs[:, b * HID_F : (b + 1) * HID_F])

    nc.compile()
    return nc


def _run(inputs, trace=False, tmpdir=None):
    from concourse.bass_utils import run_bass_kernel_spmd

    x = np.asarray(inputs["x"], np.float32)
    src = np.asarray(inputs["src"])
    dst = np.asarray(inputs["dst"])
    gspec, pos, percore = _prep(x, src, dst)
    b1 = np.asarray(inputs["b1"], np.float32)
    b2 = np.asarray(inputs["b2"], np.float32)
    has_b1 = bool(np.any(b1))
    has_b2 = bool(np.any(b2))

    key = (gspec, has_b1, has_b2)
    if key not in _cache:
        _cache[key] = _build(gspec, has_b1, has_b2)
    nc = _cache[key]

    shared = {
        "w1n": np.asarray(inputs["W1_neigh"], np.float32),
        "w1s": np.asarray(inputs["W1_self"], np.float32),
        "w2n": np.asarray(inputs["W2_neigh"], np.float32),
        "w2s": np.asarray(inputs["W2_self"], np.float32),
        "b1r": np.broadcast_to(b1, (P, HID_F)).copy(),
        "b2r": np.broadcast_to(b2, (P, OUT_F)).copy(),
    }
    in_maps = []
    for c in range(CORES):
        xT, idx_t, m_arr, invd_exp = percore[c]
        mp = dict(shared)
        mp.update({"xT": xT, "idxs": idx_t, "mpar": m_arr, "invd": invd_exp})
        in_maps.append(mp)

    res = run_bass_kernel_spmd(nc, in_maps, list(range(CORES)),
                               trace=trace, tmpdir=tmpdir)
    h2_new = np.concatenate([res.results[c]["out"] for c in range(CORES)], axis=0)
    out = h2_new[pos]
    return out.astype(np.float32), res


def kernel(**inputs) -> np.ndarray:
    out, _ = _run(inputs, trace=False)
    return out


# revision 25
# speedup vs baseline: 1.6979x; 1.1056x over previous
"""GraphSAGE 2-layer kernel for 8 Trainium2 NeuronCores.

Strategy (dst-partitioned, batched ucode gather + DVE select/fold reduction):
  - Nodes dealt round-robin by global in-degree rank to the 8 cores, so every
    core's rank-r row has ~the same degree -> per-block run widths are
    near-uniform and padding is small (~5%).
  - Pre-projection before the gather: p = h @ W_neigh per-core, AllGather'd
    to a bf16 [NPAD, 64] table in DRAM. The gather reads it through a
    [NPAD/2, 128] pair view: one 256B gather row = nodes (2q, 2q+1), so the
    int16 gather index q = src>>1 covers the whole table in one window.
  - Aggregation per block-group: one dma_gather pulls all edge tokens
    (token i -> partition i%128, column i//128; layout [128, nblk, G, 128]),
    a 3-op DVE select picks the src&1 half per token (f32 out), and a
    log-depth in-place strided DVE fold sums each row's G-token run, with the
    last step written straight into the agg columns. Pad tokens point at an
    all-zero pair row.
  - h = relu(x @ W_self + inv_deg * agg + b) with full-width in-place DVE ops
    and one scalar-engine relu; layer 2 identical (same graph -> same index
    streams) with an on-chip PE transpose of h1.
"""

import numpy as np

N = 50000
E = 800000
IN_F, HID_F, OUT_F = 128, 64, 64
CORES = 8
P = 128
NPC = N // CORES          # 6250 real nodes per core
NB_C = 49                 # dst blocks per core
R = NB_C * P              # 6272 padded rows per core
NPAD = CORES * R          # 50176
W = NB_C * HID_F          # 3136 full-width columns
ZQ = (NPC) // 2           # 3125: pair (6250, 6251) = core 0 pad rows, all-zero
CH_COLS = 56              # max token-columns per gather (7168 tokens)

_cache = {}


def _prep(x, src, dst):
    """Host-side: degree-deal nodes to cores, build grouped token streams."""
    src = np.asarray(src).astype(np.int64)
    dst = np.asarray(dst).astype(np.int64)
    deg = np.bincount(dst, minlength=N)

    order = np.argsort(-deg, kind="stable")
    pos = np.empty(N, np.int64)
    i = np.arange(N)
    pos[order] = (i % CORES) * R + (i // CORES)

    ndst = pos[dst]
    nsrc = pos[src]
    idxq_all = (nsrc >> 1).astype(np.int16)
    m_all = (nsrc & 1).astype(np.float32)

    # per-block widths D_j (max over cores); deal => profile ~identical
    degq = np.zeros(NPAD, np.int64)
    degq[pos] = deg
    D = degq.reshape(CORES, NB_C, P).max(axis=(0, 2))
    D = np.maximum(D, 1)

    assert D.max() <= CH_COLS, f"max run width {D.max()} exceeds {CH_COLS}"
    groups = []  # (j0, nblk, G)
    j = 0
    while j < NB_C:
        j0 = j
        g = 0
        while j < NB_C:
            if j > j0 and max(g, int(D[j])) * (j - j0 + 1) > CH_COLS:
                break
            g = max(g, int(D[j]))
            j += 1
        groups.append((j0, j - j0, g))
    gspec = tuple(groups)
    total_cols = sum(nb * g for (_, nb, g) in gspec)

    inv_deg_new = np.ones(NPAD, np.float32)
    vmask = np.zeros(NPAD, bool)
    vmask[pos] = True
    inv_deg_new[vmask] = (1.0 / np.maximum(degq[vmask], 1)).astype(np.float32)
    xp = np.zeros((NPAD, IN_F), np.float32)
    xp[pos] = x

    percore = []
    for c in range(CORES):
        sel = (ndst >= c * R) & (ndst < (c + 1) * R)
        r_e = ndst[sel] - c * R
        q_e = idxq_all[sel]
        pm_e = m_all[sel]
        o = np.argsort(r_e, kind="stable")
        r_s = r_e[o]
        q_s = q_e[o]
        m_s = pm_e[o]
        cnt = np.bincount(r_s, minlength=R)
        st = np.zeros(R + 1, np.int64)
        np.cumsum(cnt, out=st[1:])
        t_s = np.arange(r_s.size) - st[r_s]

        jj_s = r_s // P
        pp_s = r_s % P

        idx_stream = np.full(total_cols * P, ZQ, np.int16)
        m_arr = np.zeros((P, total_cols), np.float32)
        colbase = 0
        for (j0, nblk, G) in gspec:
            msel = (jj_s >= j0) & (jj_s < j0 + nblk)
            col = colbase + (jj_s[msel] - j0) * G + t_s[msel]
            idx_stream[col * P + pp_s[msel]] = q_s[msel]
            m_arr[pp_s[msel], col] = m_s[msel]
            colbase += nblk * G
        n_all = total_cols * P
        wrapped = np.zeros((16, n_all // 16), np.int16)
        wrapped[np.arange(n_all) % 16, np.arange(n_all) // 16] = idx_stream
        idx_t = np.tile(wrapped, (8, 1))

        iv = inv_deg_new[c * R : (c + 1) * R]
        invd_exp = np.repeat(iv.reshape(NB_C, P).T[:, :, None], HID_F,
                             axis=2).reshape(P, W)
        xT = xp[c * R : (c + 1) * R].T.astype(np.float32).copy()
        percore.append((xT, idx_t, m_arr.astype(np.float32),
                        np.ascontiguousarray(invd_exp)))
    return gspec, pos, percore


def _build(gspec, has_b1, has_b2):
    """Build + compile the SPMD bass program (uniform across cores)."""
    import concourse.bacc as bacc
    import concourse.bass as bass
    import concourse.mybir as mybir
    import concourse.tile as tile
    from concourse import library_config

    f32 = mybir.dt.float32
    bf16 = mybir.dt.bfloat16
    i16 = mybir.dt.int16
    TOTC = sum(nb * g for (_, nb, g) in gspec)
    IDXC = TOTC * P // 16

    nc = bacc.Bacc("TRN2", target_bir_lowering=False, debug=False,
                   num_devices=CORES, num_swdge_queues=4)

    xT_d = nc.dram_tensor("xT", [P, R], f32, kind="ExternalInput")
    idx_d = nc.dram_tensor("idxs", [P, IDXC], i16, kind="ExternalInput")
    m_d = nc.dram_tensor("mpar", [P, TOTC], f32, kind="ExternalInput")
    invd_d = nc.dram_tensor("invd", [P, W], f32, kind="ExternalInput")
    w1n_d = nc.dram_tensor("w1n", [IN_F, HID_F], f32, kind="ExternalInput")
    w1s_d = nc.dram_tensor("w1s", [IN_F, HID_F], f32, kind="ExternalInput")
    w2n_d = nc.dram_tensor("w2n", [HID_F, OUT_F], f32, kind="ExternalInput")
    w2s_d = nc.dram_tensor("w2s", [HID_F, OUT_F], f32, kind="ExternalInput")
    b1_d = nc.dram_tensor("b1r", [P, HID_F], f32, kind="ExternalInput")
    b2_d = nc.dram_tensor("b2r", [P, OUT_F], f32, kind="ExternalInput")
    out_d = nc.dram_tensor("out", [R, OUT_F], f32, kind="ExternalOutput")

    cc_in1 = nc.dram_tensor("cc_in1", [R, HID_F], bf16)
    cc_out1 = nc.dram_tensor("cc_out1", [NPAD, HID_F], bf16)
    cc_in2 = nc.dram_tensor("cc_in2", [R, HID_F], bf16)
    cc_out2 = nc.dram_tensor("cc_out2", [NPAD, HID_F], bf16)

    groups_rg = [list(range(CORES))]
    mul = mybir.AluOpType.mult
    sub = mybir.AluOpType.subtract
    relu = mybir.ActivationFunctionType.Relu

    from concourse.masks import make_identity

    with tile.TileContext(nc) as tc:
        with (
            tc.tile_pool(name="pers", bufs=1) as pers,
            tc.tile_pool(name="stage", bufs=4) as stage,
            tc.tile_pool(name="graw", bufs=5) as graw_pool,
            tc.tile_pool(name="gsel", bufs=2) as gsel_pool,
            tc.tile_pool(name="pproj", bufs=2, space="PSUM") as pproj,
            tc.tile_pool(name="pself", bufs=2, space="PSUM") as pself,
            tc.tile_pool(name="ptr", bufs=2, space="PSUM") as ptr_pool,
        ):
            nc.gpsimd.load_library(library_config.mlp)
            xT = pers.tile([P, R], f32)
            nc.sync.dma_start(out=xT[:], in_=xT_d[:, :])
            idxs = pers.tile([P, IDXC], i16)
            nc.sync.dma_start(out=idxs[:], in_=idx_d[:, :])
            mpar = pers.tile([P, TOTC], f32)
            nc.sync.dma_start(out=mpar[:], in_=m_d[:, :])
            invd = pers.tile([P, W], f32)
            nc.sync.dma_start(out=invd[:], in_=invd_d[:, :])
            w1n = pers.tile([IN_F, HID_F], f32)
            nc.sync.dma_start(out=w1n[:], in_=w1n_d[:, :])
            w1s = pers.tile([IN_F, HID_F], f32)
            nc.sync.dma_start(out=w1s[:], in_=w1s_d[:, :])
            w2n = pers.tile([HID_F, OUT_F], f32)
            nc.sync.dma_start(out=w2n[:], in_=w2n_d[:, :])
            w2s = pers.tile([HID_F, OUT_F], f32)
            nc.sync.dma_start(out=w2s[:], in_=w2s_d[:, :])
            b1r = pers.tile([P, HID_F], f32)
            if has_b1:
                nc.sync.dma_start(out=b1r[:], in_=b1_d[:, :])
            b2r = pers.tile([P, OUT_F], f32)
            if has_b2:
                nc.sync.dma_start(out=b2r[:], in_=b2_d[:, :])
            ident = pers.tile([P, P], f32)
            make_identity(nc, ident[:])
            h1 = pers.tile([P, W], f32)
            h1T = pers.tile([HID_F, R], f32)
            agg = pers.tile([P, W], f32)
            hs = pers.tile([P, W], f32)

            def proj_blocks(lhsT_of, w, cc_in):
                for b in range(NB_C):
                    ps = pproj.tile([P, HID_F], f32, tag="proj")
                    nc.tensor.matmul(out=ps[:], lhsT=lhsT_of(b), rhs=w[:],
                                     start=True, stop=True)
                    t = stage.tile([P, HID_F], bf16, tag="proj_sb")
                    nc.vector.tensor_copy(out=t[:], in_=ps[:])
                    nc.sync.dma_start(out=cc_in[b * P : (b + 1) * P, :], in_=t[:])

            def self_blocks(lhsT_of, w):
                for b in range(NB_C):
                    ps = pself.tile([P, HID_F], f32, tag="self")
                    nc.tensor.matmul(out=ps[:], lhsT=lhsT_of(b), rhs=w[:],
                                     start=True, stop=True)
                    nc.vector.tensor_copy(out=hs[:, b * HID_F : (b + 1) * HID_F],
                                          in_=ps[:])

            def agg_passes(cc_out):
                ccv = cc_out[:, :].rearrange("(q two) f -> q (two f)", two=2)
                colbase = 0
                qn = 0
                for (j0, nblk, G) in gspec:
                    ncols = nblk * G
                    n_tok = ncols * P
                    g = graw_pool.tile([P, CH_COLS, 2 * HID_F], bf16, tag="g")
                    nc.gpsimd.dma_gather(
                        g[:, 0:ncols, :], ccv,
                        idxs[:, colbase * P // 16 : (colbase + ncols) * P // 16],
                        n_tok, n_tok, 2 * HID_F, single_packet=False,
                        queue_num=qn)
                    qn = (qn + 1) % 4
                    lo = g[:, 0:ncols, 0:HID_F]
                    hi = g[:, 0:ncols, HID_F : 2 * HID_F]
                    mB = mpar[:, colbase : colbase + ncols].unsqueeze(2) \
                        .to_broadcast([P, ncols, HID_F])
                    s = gsel_pool.tile([P, CH_COLS, HID_F], bf16, tag="s")
                    sc = s[:, 0:ncols, :]
                    sv = sc.rearrange("p (nb gw) f -> p nb gw f",
                                      nb=nblk, gw=G)
                    nc.vector.tensor_tensor(out=sc, in0=hi, in1=lo, op=sub)
                    nc.vector.tensor_tensor(out=sc, in0=sc, in1=mB, op=mul)
                    nc.vector.tensor_add(out=sc, in0=sc, in1=lo)
                    aggv = agg[:, j0 * HID_F : (j0 + nblk) * HID_F].rearrange(
                        "p (nb f) -> p nb f", nb=nblk)
                    D = G
                    while D > 2:
                        h = D // 2
                        nc.vector.tensor_add(
                            out=sv[:, :, 0:h, :], in0=sv[:, :, 0:h, :],
                            in1=sv[:, :, D - h : D, :])
                        D = D - h
                    if D == 2:
                        nc.vector.tensor_add(out=aggv, in0=sv[:, :, 0, :],
                                             in1=sv[:, :, 1, :])
                    else:
                        nc.vector.tensor_copy(out=aggv, in_=sv[:, :, 0, :])
                    colbase += ncols

            def combine(has_b, br, out_tile):
                nc.vector.tensor_tensor(out=agg[:], in0=agg[:], in1=invd[:],
                                        op=mul)
                nc.vector.tensor_add(out=agg[:], in0=agg[:], in1=hs[:])
                if has_b:
                    for b in range(NB_C):
                        nc.vector.tensor_add(
                            out=agg[:, b * HID_F : (b + 1) * HID_F],
                            in0=agg[:, b * HID_F : (b + 1) * HID_F], in1=br[:])
                nc.scalar.activation(out=out_tile[:], in_=agg[:], func=relu)

            NPADROWS = R - NPC  # 22 pad rows: block 48, partitions 106..127

            # ---- layer 1
            proj_blocks(lambda b: xT[:, b * P : (b + 1) * P], w1n, cc_in1)
            nc.gpsimd.collective_compute(
                "AllGather", mybir.AluOpType.bypass, replica_groups=groups_rg,
                ins=[cc_in1.ap().opt()], outs=[cc_out1.ap().opt()])
            self_blocks(lambda b: xT[:, b * P : (b + 1) * P], w1s)
            agg_passes(cc_out1)
            combine(has_b1, b1r, h1)
            if has_b1:
                # keep pad rows zero so the zero pair-row stays zero in layer 2
                nc.vector.memset(
                    h1[P - NPADROWS : P, (NB_C - 1) * HID_F : NB_C * HID_F], 0.0)

            # h1 -> h1T (PE transpose) + proj2 -> cc_in2
            for b in range(NB_C):
                pt = ptr_pool.tile([HID_F, P], f32, tag="tr")
                nc.tensor.transpose(out=pt[:],
                                    in_=h1[:, b * HID_F : (b + 1) * HID_F],
                                    identity=ident[:])
                nc.vector.tensor_copy(out=h1T[:, b * P : (b + 1) * P], in_=pt[:])
                ps = pproj.tile([P, HID_F], f32, tag="proj")
                nc.tensor.matmul(out=ps[:], lhsT=h1T[:, b * P : (b + 1) * P],
                                 rhs=w2n[:], start=True, stop=True)
                t = stage.tile([P, HID_F], bf16, tag="proj_sb")
                nc.vector.tensor_copy(out=t[:], in_=ps[:])
                nc.sync.dma_start(out=cc_in2[b * P : (b + 1) * P, :], in_=t[:])

            # ---- layer 2
            nc.gpsimd.collective_compute(
                "AllGather", mybir.AluOpType.bypass, replica_groups=groups_rg,
                ins=[cc_in2.ap().opt()], outs=[cc_out2.ap().opt()])
            self_blocks(lambda b: h1T[:, b * P : (b + 1) * P], w2s)
            agg_passes(cc_out2)
            combine(has_b2, b2r, hs)
            for b in range(NB_C):
                nc.sync.dma_start(out=out_d[b * P : (b + 1) * P, :],
                                  in_=hs[:, b * HID_F : (b + 1) * HID_F])

    nc.compile()
    return nc


def _run(inputs, trace=False, tmpdir=None):
    from concourse.bass_utils import run_bass_kernel_spmd

    x = np.asarray(inputs["x"], np.float32)
    src = np.asarray(inputs["src"])
    dst = np.asarray(inputs["dst"])
    gspec, pos, percore = _prep(x, src, dst)
    b1 = np.asarray(inputs["b1"], np.float32)
    b2 = np.asarray(inputs["b2"], np.float32)
    has_b1 = bool(np.any(b1))
    has_b2 = bool(np.any(b2))

    key = (gspec, has_b1, has_b2)
    if key not in _cache:
        _cache[key] = _build(gspec, has_b1, has_b2)
    nc = _cache[key]

    shared = {
        "w1n": np.asarray(inputs["W1_neigh"], np.float32),
        "w1s": np.asarray(inputs["W1_self"], np.float32),
        "w2n": np.asarray(inputs["W2_neigh"], np.float32),
        "w2s": np.asarray(inputs["W2_self"], np.float32),
        "b1r": np.broadcast_to(b1, (P, HID_F)).copy(),
        "b2r": np.broadcast_to(b2, (P, OUT_F)).copy(),
    }
    in_maps = []
    for c in range(CORES):
        xT, idx_t, m_arr, invd_exp = percore[c]
        mp = dict(shared)
        mp.update({"xT": xT, "idxs": idx_t, "mpar": m_arr, "invd": invd_exp})
        in_maps.append(mp)

    res = run_bass_kernel_spmd(nc, in_maps, list(range(CORES)),
                               trace=trace, tmpdir=tmpdir)
    h2_new = np.concatenate([res.results[c]["out"] for c in range(CORES)], axis=0)
    out = h2_new[pos]
    return out.astype(np.float32), res


def kernel(**inputs) -> np.ndarray:
    out, _ = _run(inputs, trace=False)
    return out


# revision 26
# speedup vs baseline: 1.8527x; 1.0912x over previous
"""GraphSAGE 2-layer kernel for 8 Trainium2 NeuronCores.

Strategy (dst-partitioned, batched ucode gather + DVE select/fold reduction):
  - Nodes dealt round-robin by global in-degree rank to the 8 cores, so every
    core's rank-r row has ~the same degree -> per-block run widths are
    near-uniform and padding is small (~5%).
  - Pre-projection before the gather: p = h @ W_neigh per-core, AllGather'd
    to a bf16 [NPAD, 64] table in DRAM. The gather reads it through a
    [NPAD/2, 128] pair view: one 256B gather row = nodes (2q, 2q+1), so the
    int16 gather index q = src>>1 covers the whole table in one window.
  - Aggregation per block-group: one dma_gather pulls all edge tokens
    (token i -> partition i%128, column i//128; layout [128, nblk, G, 128]),
    a 3-op DVE select picks the src&1 half per token (f32 out), and a
    log-depth in-place strided DVE fold sums each row's G-token run, with the
    last step written straight into the agg columns. Pad tokens point at an
    all-zero pair row.
  - h = relu(x @ W_self + inv_deg * agg + b) with full-width in-place DVE ops
    and one scalar-engine relu; layer 2 identical (same graph -> same index
    streams) with an on-chip PE transpose of h1.
"""

import numpy as np

N = 50000
E = 800000
IN_F, HID_F, OUT_F = 128, 64, 64
CORES = 8
P = 128
NPC = N // CORES          # 6250 real nodes per core
NB_C = 49                 # dst blocks per core
R = NB_C * P              # 6272 padded rows per core
NPAD = CORES * R          # 50176
W = NB_C * HID_F          # 3136 full-width columns
ZQ = (NPC) // 2           # 3125: pair (6250, 6251) = core 0 pad rows, all-zero
CH_COLS = 48              # max token-columns per gather (6144 tokens)

_cache = {}


def _prep(x, src, dst):
    """Host-side: degree-deal nodes to cores, build grouped token streams."""
    src = np.asarray(src).astype(np.int64)
    dst = np.asarray(dst).astype(np.int64)
    deg = np.bincount(dst, minlength=N)

    order = np.argsort(-deg, kind="stable")
    pos = np.empty(N, np.int64)
    i = np.arange(N)
    pos[order] = (i % CORES) * R + (i // CORES)

    ndst = pos[dst]
    nsrc = pos[src]
    idxq_all = (nsrc >> 1).astype(np.int16)
    m_all = (nsrc & 1).astype(np.float32)

    # per-block widths D_j (max over cores); deal => profile ~identical
    degq = np.zeros(NPAD, np.int64)
    degq[pos] = deg
    D = degq.reshape(CORES, NB_C, P).max(axis=(0, 2))
    D = np.maximum(D, 1)

    assert D.max() <= CH_COLS, f"max run width {D.max()} exceeds {CH_COLS}"
    groups = []  # (j0, nblk, G)
    j = 0
    while j < NB_C:
        j0 = j
        g = 0
        while j < NB_C:
            if j > j0 and max(g, int(D[j])) * (j - j0 + 1) > CH_COLS:
                break
            g = max(g, int(D[j]))
            j += 1
        groups.append((j0, j - j0, g))
    gspec = tuple(groups)
    total_cols = sum(nb * g for (_, nb, g) in gspec)

    inv_deg_new = np.ones(NPAD, np.float32)
    vmask = np.zeros(NPAD, bool)
    vmask[pos] = True
    inv_deg_new[vmask] = (1.0 / np.maximum(degq[vmask], 1)).astype(np.float32)
    xp = np.zeros((NPAD, IN_F), np.float32)
    xp[pos] = x

    percore = []
    for c in range(CORES):
        sel = (ndst >= c * R) & (ndst < (c + 1) * R)
        r_e = ndst[sel] - c * R
        q_e = idxq_all[sel]
        pm_e = m_all[sel]
        o = np.argsort(r_e, kind="stable")
        r_s = r_e[o]
        q_s = q_e[o]
        m_s = pm_e[o]
        cnt = np.bincount(r_s, minlength=R)
        st = np.zeros(R + 1, np.int64)
        np.cumsum(cnt, out=st[1:])
        t_s = np.arange(r_s.size) - st[r_s]

        jj_s = r_s // P
        pp_s = r_s % P

        idx_stream = np.full(total_cols * P, ZQ, np.int16)
        m_arr = np.zeros((P, total_cols), np.float32)
        colbase = 0
        for (j0, nblk, G) in gspec:
            msel = (jj_s >= j0) & (jj_s < j0 + nblk)
            col = colbase + (jj_s[msel] - j0) * G + t_s[msel]
            idx_stream[col * P + pp_s[msel]] = q_s[msel]
            m_arr[pp_s[msel], col] = m_s[msel]
            colbase += nblk * G
        n_all = total_cols * P
        wrapped = np.zeros((16, n_all // 16), np.int16)
        wrapped[np.arange(n_all) % 16, np.arange(n_all) // 16] = idx_stream
        idx_t = np.tile(wrapped, (8, 1))

        iv = inv_deg_new[c * R : (c + 1) * R]
        invd_exp = np.repeat(iv.reshape(NB_C, P).T[:, :, None], HID_F,
                             axis=2).reshape(P, W)
        xT = xp[c * R : (c + 1) * R].T.astype(np.float32).copy()
        percore.append((xT, idx_t, m_arr.astype(np.float32),
                        np.ascontiguousarray(invd_exp)))
    return gspec, pos, percore


def _build(gspec, has_b1, has_b2):
    """Build + compile the SPMD bass program (uniform across cores)."""
    import concourse.bacc as bacc
    import concourse.bass as bass
    import concourse.mybir as mybir
    import concourse.tile as tile
    from concourse import library_config

    f32 = mybir.dt.float32
    bf16 = mybir.dt.bfloat16
    i16 = mybir.dt.int16
    TOTC = sum(nb * g for (_, nb, g) in gspec)
    IDXC = TOTC * P // 16

    nc = bacc.Bacc("TRN2", target_bir_lowering=False, debug=False,
                   num_devices=CORES, num_swdge_queues=4)

    xT_d = nc.dram_tensor("xT", [P, R], f32, kind="ExternalInput")
    idx_d = nc.dram_tensor("idxs", [P, IDXC], i16, kind="ExternalInput")
    m_d = nc.dram_tensor("mpar", [P, TOTC], f32, kind="ExternalInput")
    invd_d = nc.dram_tensor("invd", [P, W], f32, kind="ExternalInput")
    w1n_d = nc.dram_tensor("w1n", [IN_F, HID_F], f32, kind="ExternalInput")
    w1s_d = nc.dram_tensor("w1s", [IN_F, HID_F], f32, kind="ExternalInput")
    w2n_d = nc.dram_tensor("w2n", [HID_F, OUT_F], f32, kind="ExternalInput")
    w2s_d = nc.dram_tensor("w2s", [HID_F, OUT_F], f32, kind="ExternalInput")
    b1_d = nc.dram_tensor("b1r", [P, HID_F], f32, kind="ExternalInput")
    b2_d = nc.dram_tensor("b2r", [P, OUT_F], f32, kind="ExternalInput")
    out_d = nc.dram_tensor("out", [R, OUT_F], f32, kind="ExternalOutput")

    cc_in1 = nc.dram_tensor("cc_in1", [R, HID_F], bf16)
    cc_out1 = nc.dram_tensor("cc_out1", [NPAD, HID_F], bf16)
    cc_in2 = nc.dram_tensor("cc_in2", [R, HID_F], bf16)
    cc_out2 = nc.dram_tensor("cc_out2", [NPAD, HID_F], bf16)

    groups_rg = [list(range(CORES))]
    mul = mybir.AluOpType.mult
    sub = mybir.AluOpType.subtract
    relu = mybir.ActivationFunctionType.Relu

    from concourse.masks import make_identity

    with tile.TileContext(nc) as tc:
        with (
            tc.tile_pool(name="pers", bufs=1) as pers,
            tc.tile_pool(name="stage", bufs=4) as stage,
            tc.tile_pool(name="graw", bufs=6) as graw_pool,
            tc.tile_pool(name="gsel", bufs=2) as gsel_pool,
            tc.tile_pool(name="pproj", bufs=2, space="PSUM") as pproj,
            tc.tile_pool(name="pself", bufs=2, space="PSUM") as pself,
            tc.tile_pool(name="ptr", bufs=2, space="PSUM") as ptr_pool,
        ):
            nc.gpsimd.load_library(library_config.mlp)
            xT = pers.tile([P, R], f32)
            nc.sync.dma_start(out=xT[:], in_=xT_d[:, :])
            idxs = pers.tile([P, IDXC], i16)
            nc.sync.dma_start(out=idxs[:], in_=idx_d[:, :])
            mpar = pers.tile([P, TOTC], f32)
            nc.sync.dma_start(out=mpar[:], in_=m_d[:, :])
            invd = pers.tile([P, W], f32)
            nc.sync.dma_start(out=invd[:], in_=invd_d[:, :])
            w1n = pers.tile([IN_F, HID_F], f32)
            nc.sync.dma_start(out=w1n[:], in_=w1n_d[:, :])
            w1s = pers.tile([IN_F, HID_F], f32)
            nc.sync.dma_start(out=w1s[:], in_=w1s_d[:, :])
            w2n = pers.tile([HID_F, OUT_F], f32)
            nc.sync.dma_start(out=w2n[:], in_=w2n_d[:, :])
            w2s = pers.tile([HID_F, OUT_F], f32)
            nc.sync.dma_start(out=w2s[:], in_=w2s_d[:, :])
            b1r = pers.tile([P, HID_F], f32)
            if has_b1:
                nc.sync.dma_start(out=b1r[:], in_=b1_d[:, :])
            b2r = pers.tile([P, OUT_F], f32)
            if has_b2:
                nc.sync.dma_start(out=b2r[:], in_=b2_d[:, :])
            ident = pers.tile([P, P], f32)
            make_identity(nc, ident[:])
            h1 = pers.tile([P, W], f32)
            h1T = pers.tile([HID_F, R], f32)
            agg = pers.tile([P, W], f32)
            hs = pers.tile([P, W], f32)

            def proj_blocks(lhsT_of, w, cc_in):
                for b in range(NB_C):
                    ps = pproj.tile([P, HID_F], f32, tag="proj")
                    nc.tensor.matmul(out=ps[:], lhsT=lhsT_of(b), rhs=w[:],
                                     start=True, stop=True)
                    t = stage.tile([P, HID_F], bf16, tag="proj_sb")
                    nc.vector.tensor_copy(out=t[:], in_=ps[:])
                    nc.sync.dma_start(out=cc_in[b * P : (b + 1) * P, :], in_=t[:])

            def self_blocks(lhsT_of, w):
                for b in range(NB_C):
                    ps = pself.tile([P, HID_F], f32, tag="self")
                    nc.tensor.matmul(out=ps[:], lhsT=lhsT_of(b), rhs=w[:],
                                     start=True, stop=True)
                    nc.vector.tensor_copy(out=hs[:, b * HID_F : (b + 1) * HID_F],
                                          in_=ps[:])

            def agg_passes(cc_out):
                ccv = cc_out[:, :].rearrange("(q two) f -> q (two f)", two=2)
                colbase = 0
                qn = 0
                for (j0, nblk, G) in gspec:
                    ncols = nblk * G
                    n_tok = ncols * P
                    g = graw_pool.tile([P, CH_COLS, 2 * HID_F], bf16, tag="g")
                    nc.gpsimd.dma_gather(
                        g[:, 0:ncols, :], ccv,
                        idxs[:, colbase * P // 16 : (colbase + ncols) * P // 16],
                        n_tok, n_tok, 2 * HID_F, single_packet=False,
                        queue_num=qn)
                    qn = (qn + 1) % 4
                    lo = g[:, 0:ncols, 0:HID_F]
                    hi = g[:, 0:ncols, HID_F : 2 * HID_F]
                    mB = mpar[:, colbase : colbase + ncols].unsqueeze(2) \
                        .to_broadcast([P, ncols, HID_F])
                    s = gsel_pool.tile([P, CH_COLS, HID_F], bf16, tag="s")
                    sc = s[:, 0:ncols, :]
                    sv = sc.rearrange("p (nb gw) f -> p nb gw f",
                                      nb=nblk, gw=G)
                    nc.vector.tensor_tensor(out=sc, in0=hi, in1=lo, op=sub)
                    nc.vector.tensor_tensor(out=sc, in0=sc, in1=mB, op=mul)
                    nc.vector.tensor_add(out=sc, in0=sc, in1=lo)
                    aggv = agg[:, j0 * HID_F : (j0 + nblk) * HID_F].rearrange(
                        "p (nb f) -> p nb f", nb=nblk)
                    D = G
                    while D > 2:
                        h = D // 2
                        nc.vector.tensor_add(
                            out=sv[:, :, 0:h, :], in0=sv[:, :, 0:h, :],
                            in1=sv[:, :, D - h : D, :])
                        D = D - h
                    if D == 2:
                        nc.vector.tensor_add(out=aggv, in0=sv[:, :, 0, :],
                                             in1=sv[:, :, 1, :])
                    else:
                        nc.vector.tensor_copy(out=aggv, in_=sv[:, :, 0, :])
                    colbase += ncols

            def combine(has_b, br, out_tile):
                nc.vector.tensor_tensor(out=agg[:], in0=agg[:], in1=invd[:],
                                        op=mul)
                nc.vector.tensor_add(out=agg[:], in0=agg[:], in1=hs[:])
                if has_b:
                    for b in range(NB_C):
                        nc.vector.tensor_add(
                            out=agg[:, b * HID_F : (b + 1) * HID_F],
                            in0=agg[:, b * HID_F : (b + 1) * HID_F], in1=br[:])
                nc.scalar.activation(out=out_tile[:], in_=agg[:], func=relu)

            NPADROWS = R - NPC  # 22 pad rows: block 48, partitions 106..127

            # ---- layer 1
            proj_blocks(lambda b: xT[:, b * P : (b + 1) * P], w1n, cc_in1)
            nc.gpsimd.collective_compute(
                "AllGather", mybir.AluOpType.bypass, replica_groups=groups_rg,
                ins=[cc_in1.ap().opt()], outs=[cc_out1.ap().opt()])
            self_blocks(lambda b: xT[:, b * P : (b + 1) * P], w1s)
            agg_passes(cc_out1)
            combine(has_b1, b1r, h1)
            if has_b1:
                # keep pad rows zero so the zero pair-row stays zero in layer 2
                nc.vector.memset(
                    h1[P - NPADROWS : P, (NB_C - 1) * HID_F : NB_C * HID_F], 0.0)

            # h1 -> h1T (PE transpose) + proj2 -> cc_in2
            for b in range(NB_C):
                pt = ptr_pool.tile([HID_F, P], f32, tag="tr")
                nc.tensor.transpose(out=pt[:],
                                    in_=h1[:, b * HID_F : (b + 1) * HID_F],
                                    identity=ident[:])
                nc.vector.tensor_copy(out=h1T[:, b * P : (b + 1) * P], in_=pt[:])
                ps = pproj.tile([P, HID_F], f32, tag="proj")
                nc.tensor.matmul(out=ps[:], lhsT=h1T[:, b * P : (b + 1) * P],
                                 rhs=w2n[:], start=True, stop=True)
                t = stage.tile([P, HID_F], bf16, tag="proj_sb")
                nc.vector.tensor_copy(out=t[:], in_=ps[:])
                nc.sync.dma_start(out=cc_in2[b * P : (b + 1) * P, :], in_=t[:])

            # ---- layer 2
            nc.gpsimd.collective_compute(
                "AllGather", mybir.AluOpType.bypass, replica_groups=groups_rg,
                ins=[cc_in2.ap().opt()], outs=[cc_out2.ap().opt()])
            self_blocks(lambda b: h1T[:, b * P : (b + 1) * P], w2s)
            agg_passes(cc_out2)
            combine(has_b2, b2r, hs)
            for b in range(NB_C):
                nc.sync.dma_start(out=out_d[b * P : (b + 1) * P, :],
                                  in_=hs[:, b * HID_F : (b + 1) * HID_F])

    nc.compile()
    return nc


def _run(inputs, trace=False, tmpdir=None):
    from concourse.bass_utils import run_bass_kernel_spmd

    x = np.asarray(inputs["x"], np.float32)
    src = np.asarray(inputs["src"])
    dst = np.asarray(inputs["dst"])
    gspec, pos, percore = _prep(x, src, dst)
    b1 = np.asarray(inputs["b1"], np.float32)
    b2 = np.asarray(inputs["b2"], np.float32)
    has_b1 = bool(np.any(b1))
    has_b2 = bool(np.any(b2))

    key = (gspec, has_b1, has_b2)
    if key not in _cache:
        _cache[key] = _build(gspec, has_b1, has_b2)
    nc = _cache[key]

    shared = {
        "w1n": np.asarray(inputs["W1_neigh"], np.float32),
        "w1s": np.asarray(inputs["W1_self"], np.float32),
        "w2n": np.asarray(inputs["W2_neigh"], np.float32),
        "w2s": np.asarray(inputs["W2_self"], np.float32),
        "b1r": np.broadcast_to(b1, (P, HID_F)).copy(),
        "b2r": np.broadcast_to(b2, (P, OUT_F)).copy(),
    }
    in_maps = []
    for c in range(CORES):
        xT, idx_t, m_arr, invd_exp = percore[c]
        mp = dict(shared)
        mp.update({"xT": xT, "idxs": idx_t, "mpar": m_arr, "invd": invd_exp})
        in_maps.append(mp)

    res = run_bass_kernel_spmd(nc, in_maps, list(range(CORES)),
                               trace=trace, tmpdir=tmpdir)
    h2_new = np.concatenate([res.results[c]["out"] for c in range(CORES)], axis=0)
    out = h2_new[pos]
    return out.astype(np.float32), res


def kernel(**inputs) -> np.ndarray:
    out, _ = _run(inputs, trace=False)
    return out
